# revision 52
# baseline (speedup 1.0000x reference)
"""Trainium2 Bass kernel for nn_Comm_OUT (MTRNN -> Ted_Conv1d -> proj -> comm mask).

Data-parallel over N = E*S = 2048 sequences across 8 NeuronCores (256 each).
Per core, fully fused in SBUF:
  phase 0: xw = x @ Wx                      (transposed layout: H on partitions)
  phase 1+2 interleaved: 32-step MTRNN h = tanh(xw + h @ Wh + b) writing the
           hidden-state history, with the 4 parallel convs (k=1,3,5,7,
           reflect padding) consuming it as shifted matmuls paced by
           tile-pool WAR dependencies; PReLU fused into PSUM eviction
  phase 3: projection to C=64 logits per position
  phase 4: comm mask = "no end token (argmax==0) strictly before l",
           kept as a running per-position alive flag (mkl)
All matmul operands are bf16 (PSUM accumulation stays fp32): full PE rate at
any free size, fast weight load, and half the DMA/SBUF traffic.

Performance notes (sim-verified, CoreSim ~283.3us vs 297.9us baseline,
i.e. ~309us estimated HW vs 325.1us measured baseline):
  - all weight DRAM/SBUF layouts keep >=512B innermost contiguity; smaller
    innermost runs double DMA latency (read-update-write at both ends)
  - phase 0 tanh(t=0) reads xw straight from the still-open PSUM group and
    RNN step 1 keeps accumulating Wh@h0 into that same group (diagonal
    tanh-availability wavefront), removing the vector-add latency chain
    from the pipeline ramp; a dummy tanh at t~0 preloads the activation
    table during the initial DMA wait
  - reflect-padding boundary taps that land on the same source position
    are merged into single matmuls against on-chip pre-summed weights
    (~16 tap-matmuls saved, built on the idle Pool engine)
  - the last l-tile uses per-position PSUM groups and per-position DMA so
    the final drain chain is one position deep; its mask-state update is
    dead code and skipped
  - when bout == 0 (guaranteed by the input spec, runtime-checked with a
    general fallback) the projection bias add is dropped and the mask
    pipeline reads the proj PSUM directly
"""

import numpy as np

import concourse.bass as bass
import concourse.mybir as mybir
from concourse.tile import TileContext
from concourse.bass_utils import run_bass_kernel_spmd

F32 = mybir.dt.float32
F32R = mybir.dt.float32r
BF16 = mybir.dt.bfloat16
AF = mybir.ActivationFunctionType
ALU = mybir.AluOpType

E, S, L, H, D_IN, C = 32, 64, 32, 512, 1536, 64
N = E * S
NCORES = 8
NC_N = N // NCORES          # 256 rows per core
HC = H // 128               # 4 H chunks
DC = D_IN // 128            # 12 D_IN chunks
TL = 2                      # output-l positions per conv PSUM tile
RING = 32                   # full hidden-state history (no ring wraps)
KS = [1, 3, 5, 7]

_uid = [0]


def _split_excess_waits(nc, limit=1):
    """walrus in this toolchain accepts at most one sem-wait per instruction;
    move excess waits onto same-engine no-ops inserted just before."""
    for f in nc.m.functions:
        for bb in f.blocks:
            insts = bb.instructions
            i = 0
            while i < len(insts):
                inst = insts[i]
                si = inst.sync_info
                waits = list(si.on_wait) if si and si.on_wait else []
                if len(waits) > limit:
                    excess, keep = waits[:-limit], waits[-limit:]
                    inst.sync_info = mybir.SyncInfo(
                        on_wait=keep, on_update=list(si.on_update or []))
                    pos = i
                    for j in range(0, len(excess), limit):
                        _uid[0] += 1
                        nop = mybir.InstNoOp(
                            name=f"I-waitsplit-{_uid[0]}", ins=[], outs=[])
                        nop.engine = inst.engine
                        nop.bass_nofuse = True
                        nop.sync_info = mybir.SyncInfo(
                            on_wait=excess[j:j + limit], on_update=[])
                        insts.insert(pos, nop)
                        nc.register_instruction(nop, overwrite=True)
                        pos += 1
                        i += 1
                i += 1
            bb.instructions = insts


def _reflect(i):
    if i < 0:
        return -i
    if i > L - 1:
        return 2 * (L - 1) - i
    return i


def _conv_mm_plan():
    """Per (ltile, conv): two waves of (wref, kc, slot0, n_l, out_j) matmuls
    over the hidden-state history, wref = ('t', dk) for a plain tap or
    ('m', idx) for a merged boundary weight. Wave A contains work whose
    newest needed hidden state is <= step 2*lt+1 (available 3 RNN steps
    before the rest), wave B the remainder (needs <= 2*lt+4).

    At the reflect-padding edges two taps of one conv land on the same
    source position (reflect(l+o1) == reflect(l+o2)); those collapse into
    ONE matmul against a pre-summed weight w[dk1]+w[dk2] (built on-chip),
    saving ~20 tap-matmuls. The center tap never reflects and is emitted
    first with a full-tile n_l=TL run, so the PSUM group's start=True
    matmul covers the whole tile (start only overwrites the bytes it
    writes — a partial first write would accumulate onto stale PSUM).

    Returns (plans, merges) with merges = [(ci, dk1, dk2), ...]."""
    plans = {}
    merges = []
    midx = {}

    def merge_ref(ci, dks):
        key = (ci,) + tuple(sorted(dks))
        if key not in midx:
            midx[key] = len(merges)
            merges.append((ci, key[1], key[2]))
        return ('m', midx[key])

    for ci, k in enumerate(KS):
        p = (k - 1) // 2
        for lt in range(L // TL):
            l0 = TL * lt
            # per output position, group non-center taps by source
            ents = {}          # j -> list of (wref, s)
            for j in range(TL):
                src_map = {}
                for dk in range(k):
                    if dk == p:
                        continue
                    src_map.setdefault(_reflect(l0 + j + dk - p), []).append(dk)
                ents[j] = [(('t', dks[0]) if len(dks) == 1
                            else merge_ref(ci, dks), s)
                           for s, dks in sorted(src_map.items())]
            if lt == L // TL - 1:
                # last tile: one PSUM group PER output position (center tap
                # first per position) so each l's eviction/proj/mask/DMA
                # drain overlaps the other's conv matmuls at kernel end
                per_j = []
                for j in range(TL):
                    lst = [(('t', p), kc, l0 + j, 1, j) for kc in range(HC)]
                    for wref, s in ents[j]:
                        lst.extend((wref, kc, s, 1, j) for kc in range(HC))
                    per_j.append(lst)
                plans[(lt, ci)] = (None, per_j)
                continue
            # pair identical-wref entries of adjacent j with consecutive
            # sources into n_l=2 runs (interior taps)
            runs = []          # (wref, s0, n_l, out_j)
            j1 = dict(ents[1]) if TL == 2 else {}
            used = set()
            for wref, s in ents[0]:
                if j1.get(wref, None) == s + 1:
                    runs.append((wref, s, 2, 0))
                    used.add(wref)
                else:
                    runs.append((wref, s, 1, 0))
            for wref, s in ents[1]:
                if wref not in used:
                    runs.append((wref, s, 1, 1))
            # center tap first (full-tile run) so the group's start=True
            # matmul covers the whole tile
            runs.insert(0, (('t', p), l0, TL, 0))
            waves = ([], [])
            for wref, s, nl, j in runs:
                # merged weights are built on-chip on the Pool engine only
                # after the wc DMAs land (~10-14us); the early tiles' merged
                # entries go to wave B to stay clear of that
                if wref[0] == 'm' and lt <= 1:
                    wave = 1
                else:
                    wave = 0 if s + nl - 1 <= TL * lt + 1 else 1
                for kc in range(HC):
                    waves[wave].append((wref, kc, s, nl, j))
            plans[(lt, ci)] = waves
    return plans, merges


def build_nc(prelu_a: float, rep: int = 1, bias_free: bool = False):
    nc = bass.Bass()

    # all weight layouts keep >=512B innermost contiguity: DMAs with
    # innermost runs < 512B pay a 2x latency multiplier (read-update-write
    # at both SBUF and HBM), which doubled every weight-load in the
    # previous [.., HC, 128] shapes.
    xt_d = nc.declare_dram_parameter("xt", [128, DC, NC_N], BF16, isOutput=False)
    wx_d = nc.declare_dram_parameter("wx", [128, DC, H], BF16, isOutput=False)
    wh_d = nc.declare_dram_parameter("wh", [128, HC, H], BF16, isOutput=False)
    wc_d = [nc.declare_dram_parameter(f"wc{k}", [128, k, H], BF16,
                                      isOutput=False) for k in KS]
    wo_d = nc.declare_dram_parameter("wo", [128, HC * C], BF16, isOutput=False)
    bsum_d = nc.declare_dram_parameter("bsum", [128, HC], F32, isOutput=False)
    cb_d = nc.declare_dram_parameter("cb", [128, HC], F32, isOutput=False)
    bout_d = nc.declare_dram_parameter("bout", [128, C], F32, isOutput=False)
    out_d = nc.declare_dram_parameter("out", [NC_N, L, C], F32, isOutput=True)

    plans, merges = _conv_mm_plan()

    with TileContext(nc) as tc:
        with (
            tc.tile_pool(name="const", bufs=1) as cpool,
            tc.tile_pool(name="main", bufs=1) as mpool,
            tc.tile_pool(name="yt", bufs=2) as ypool,
            tc.tile_pool(name="msk", bufs=1) as kpool,
        ):
            # ---- DMA schedule across the 3 DMA-capable queues, paced so
            # phase 0 (PE consumes chunk d at ~2.5+0.43d us) never starves:
            #   sync:   wx in six 2-chunk segments (chunk pair d lands at
            #           ~2.6+0.79*(d/2) us, just ahead of consumption)
            #   gpsimd: xt in (2,4,6) segments, then wc1..wc7, then wo
            #   scalar: bsum, wh (needed at RNN t=1 ~8us), cb, bout
            wx_sb = cpool.tile([128, DC, H], BF16, tag="wx", name="wx")
            xt_sb = cpool.tile([128, DC, NC_N], BF16, tag="xt", name="xt")
            wh_sb = cpool.tile([128, HC, H], BF16, tag="wh", name="wh")
            # dummy activation in the DMA-wait dead time pulls the ~1.3us
            # ACT_TABLE_LOAD off the first real tanh's critical path
            warm = cpool.tile([128, 2], F32, tag="warm", name="warm")
            nc.vector.memset(warm[:, 0:1], 0.0)
            nc.scalar.activation(warm[:, 1:2], warm[:, 0:1], AF.Tanh)
            bsum_sb = cpool.tile([128, HC], F32, tag="bsum", name="bsum")
            nc.scalar.dma_start(out=bsum_sb[:], in_=bsum_d[:, :])
            nc.sync.dma_start(out=wx_sb[:, 0:1, :], in_=wx_d[:, 0:1, :])
            nc.gpsimd.dma_start(out=xt_sb[:, 0:1, :], in_=xt_d[:, 0:1, :])
            nc.sync.dma_start(out=wx_sb[:, 1:2, :], in_=wx_d[:, 1:2, :])
            nc.gpsimd.dma_start(out=xt_sb[:, 1:2, :], in_=xt_d[:, 1:2, :])
            nc.scalar.dma_start(out=wh_sb[:], in_=wh_d[:, :, :])
            for d0 in range(2, DC, 2):
                nc.sync.dma_start(out=wx_sb[:, d0:d0 + 2, :],
                                  in_=wx_d[:, d0:d0 + 2, :])
            nc.gpsimd.dma_start(out=xt_sb[:, 2:6, :], in_=xt_d[:, 2:6, :])
            nc.gpsimd.dma_start(out=xt_sb[:, 6:12, :], in_=xt_d[:, 6:12, :])
            cb_sb = cpool.tile([128, HC], F32, tag="cb", name="cb")
            nc.scalar.dma_start(out=cb_sb[:], in_=cb_d[:, :])
            bout_bc = None
            if not bias_free:
                bout_bc = cpool.tile([128, C], F32, tag="boutbc", name="boutbc")
                nc.scalar.dma_start(out=bout_bc[:], in_=bout_d[:, :])
            wc_sb = []
            for i, k in enumerate(KS):
                t = cpool.tile([128, k, H], BF16, tag=f"wc{k}", name=f"wc{k}")
                nc.gpsimd.dma_start(out=t[:], in_=wc_d[i][:, :, :])
                wc_sb.append(t)
            wo_sb = cpool.tile([128, HC * C], BF16, tag="wo", name="wo")
            nc.gpsimd.dma_start(out=wo_sb[:], in_=wo_d[:, :])
            # merged boundary-tap weights, built on-chip on the (otherwise
            # idle) Pool engine once the wc DMAs land; ordered so the
            # earliest-needed (small-k convs, left edge) come first
            wm_sb = None
            if merges:
                wm_sb = cpool.tile([128, len(merges), H], BF16,
                                   tag="wm", name="wm")
                for mi, (ci, dk1, dk2) in enumerate(merges):
                    nc.gpsimd.tensor_tensor(
                        wm_sb[:, mi, :], wc_sb[ci][:, dk1, :],
                        wc_sb[ci][:, dk2, :], op=ALU.add)

            def conv_w(wref, ci, kc):
                if wref[0] == 't':
                    return wc_sb[ci][:, wref[1], kc * 128:(kc + 1) * 128]
                return wm_sb[:, wref[1], kc * 128:(kc + 1) * 128]

            # ---- persistent state ----
            hs = [mpool.tile([128, RING, NC_N], BF16, tag=f"hs{m}", name=f"hs{m}")
                  for m in range(HC)]
            xw = [mpool.tile([128, NC_N], F32, tag=f"xw{m}", name=f"xw{m}")
                  for m in range(HC)]
            P = [mpool.tile([128, L, C], F32, tag=f"P{h}", name=f"P{h}")
                 for h in range(2)]
            emax = [kpool.tile([128, L], F32, tag=f"emax{h}", name=f"emax{h}")
                    for h in range(2)]
            eend = [kpool.tile([128, L], F32, tag=f"eend{h}", name=f"eend{h}")
                    for h in range(2)]
            # mkl[:, l] = "still alive at l" (no end token strictly before
            # l); written at slot l+1 so each step has no write-after-read
            # hazard with the P multiply that reads slot l
            mkl = [kpool.tile([128, L + 1], F32, tag=f"mkl{h}", name=f"mkl{h}")
                   for h in range(2)]

            def rnn_step(t, ps1):
                for m in range(HC):
                    ps = ps1.tile([128, NC_N], F32, tag="ps1", name="ps1")
                    for kc in range(HC):
                        nc.tensor.matmul(ps[:], wh_sb[:, kc, m * 128:(m + 1) * 128],
                                         hs[kc][:, (t - 1) % RING, :],
                                         start=(kc == 0), stop=(kc == HC - 1))
                    tmp = ypool.tile([128, NC_N], F32, tag="rnntmp",
                                     name="rnntmp", bufs=3)
                    nc.vector.tensor_tensor(tmp[:], ps[:], xw[m][:], op=ALU.add)
                    nc.scalar.activation(hs[m][:, t % RING, :], tmp[:], AF.Tanh,
                                         bias=bsum_sb[:, m:m + 1])

            psc_live = {}

            def conv_waveA(lt, ps2):
                tiles = []
                for ci, k in enumerate(KS):
                    psc = ps2.tile([128, TL, NC_N], F32, tag="psc", name="psc")
                    wA, wB = plans[(lt, ci)]
                    for idx, (wref, kc, s0, n_l, out_j) in enumerate(wA):
                        dst = psc[:, :, :] if n_l == TL else \
                            psc[:, out_j:out_j + 1, :]
                        nc.tensor.matmul(
                            dst, conv_w(wref, ci, kc),
                            hs[kc][:, s0:s0 + n_l, :],
                            start=(idx == 0),
                            stop=(not wB and idx == len(wA) - 1))
                    tiles.append(psc)
                psc_live[lt] = tiles

            def proj_mask(l, j, h, yts, ps3):
                """proj to C logits, then end-token mask bookkeeping.
                The P multiply (scale = mkl[:, l], which only depends on
                positions < l) runs on DVE for h=0 and the scalar engine
                for h=1 so the two halves drain in parallel. At l = L-1 the
                mask-state update is dead code (no later position consumes
                it) and is skipped. (Pool/gpsimd can't take the compare ops
                — walrus rejects non-add TensorTensor on Pool.)

                When bout == 0 (true per the input spec) the bias add is
                dropped entirely: the reductions and the mask multiply read
                the proj PSUM tile directly, removing one DVE op + sem hop
                from every position's drain chain."""
                e = nc.vector
                psp = ps3.tile([128, C], F32, tag="psp", name="psp")
                # contract kc descending: the largest conv (chunk 3) evicts
                # from PSUM first at the tile drain, so the proj's last
                # matmul waits only on the small conv1's PReLU
                for i, kc in enumerate(reversed(range(HC))):
                    nc.tensor.matmul(
                        psp[:],
                        yts[kc][:, j, h * 128:(h + 1) * 128],
                        wo_sb[:, kc * C:(kc + 1) * C],
                        start=(i == 0), stop=(i == HC - 1))
                if bias_free:
                    src = psp[:, :]
                    src_rest = psp[:, 1:]
                    src_end = psp[:, 0:1]
                else:
                    nc.vector.tensor_tensor(P[h][:, l, :], psp[:],
                                            bout_bc[:], op=ALU.add)
                    src = P[h][:, l, :]
                    src_rest = P[h][:, l, 1:]
                    src_end = P[h][:, l, 0:1]
                if l < L - 1:
                    nc.vector.tensor_reduce(
                        emax[h][:, l:l + 1], src_rest,
                        axis=mybir.AxisListType.X, op=ALU.max)
                    e.tensor_tensor(
                        eend[h][:, l:l + 1], src_end,
                        emax[h][:, l:l + 1], op=ALU.is_ge)
                    e.tensor_tensor(
                        mkl[h][:, l + 1:l + 2], mkl[h][:, l:l + 1],
                        eend[h][:, l:l + 1], op=ALU.is_gt)
                if h == 0:
                    nc.vector.tensor_scalar(
                        P[h][:, l, :], src,
                        mkl[h][:, l:l + 1], None, ALU.mult)
                else:
                    nc.scalar.activation(P[h][:, l, :], src,
                                         AF.Copy, scale=mkl[h][:, l:l + 1])

            def conv_ltile(lt, ps2, ps3):
                l0 = TL * lt
                tiles = psc_live.pop(lt)
                yts = []
                for ci, k in enumerate(KS):
                    psc = tiles[ci]
                    mms = plans[(lt, ci)][1]
                    nmm = len(mms)
                    for idx, (wref, kc, s0, n_l, out_j) in enumerate(mms):
                        dst = psc[:, :, :] if n_l == TL else \
                            psc[:, out_j:out_j + 1, :]
                        nc.tensor.matmul(
                            dst, conv_w(wref, ci, kc),
                            hs[kc][:, s0:s0 + n_l, :],
                            start=False, stop=(idx == nmm - 1))
                    yt = ypool.tile([128, TL, NC_N], BF16, tag=f"yt{ci}",
                                    name=f"yt{ci}", bufs=3)
                    nc.scalar.activation(yt[:], psc[:], AF.Prelu,
                                         bias=cb_sb[:, ci:ci + 1],
                                         alpha=float(prelu_a))
                    yts.append(yt)
                for j in range(TL):
                    for h in range(2):
                        proj_mask(l0 + j, j, h, yts, ps3)
                for h in range(2):
                    nc.sync.dma_start(
                        out=out_d[h * 128:(h + 1) * 128, l0:l0 + TL, :],
                        in_=P[h][:, l0:l0 + TL, :])

            def conv_ltile_last(ps2, ps3):
                """Last tile: per-position PSUM groups so position l0's
                whole drain (PReLU, proj, mask, DMA out) overlaps position
                l0+1's conv matmuls."""
                lt = L // TL - 1
                l0 = TL * lt
                pscs = [ps2.tile([128, TL, NC_N], F32, tag="psc", name="psc")
                        for _ in KS]
                yts = [ypool.tile([128, TL, NC_N], BF16, tag=f"yt{ci}",
                                  name=f"yt{ci}", bufs=3)
                       for ci in range(len(KS))]
                for j in range(TL):
                    # largest conv first: its (long) eviction overlaps the
                    # smaller convs' matmuls
                    for ci in reversed(range(len(KS))):
                        mms = plans[(lt, ci)][1][j]
                        nmm = len(mms)
                        for idx, (wref, kc, s0, n_l, out_j) in enumerate(mms):
                            nc.tensor.matmul(
                                pscs[ci][:, j:j + 1, :], conv_w(wref, ci, kc),
                                hs[kc][:, s0:s0 + 1, :],
                                start=(idx == 0), stop=(idx == nmm - 1))
                        nc.scalar.activation(yts[ci][:, j, :],
                                             pscs[ci][:, j, :], AF.Prelu,
                                             bias=cb_sb[:, ci:ci + 1],
                                             alpha=float(prelu_a))
                    for h in range(2):
                        proj_mask(l0 + j, j, h, yts, ps3)
                        # h=1 goes out on the scalar queue so the last two
                        # DMAs issue (and complete) in parallel
                        q = nc.sync if h == 0 else nc.scalar
                        q.dma_start(
                            out=out_d[h * 128:(h + 1) * 128,
                                      l0 + j:l0 + j + 1, :],
                            in_=P[h][:, l0 + j:l0 + j + 1, :])

            for _ in range(rep):
                with (
                    tc.tile_pool(name="ps1", bufs=2, space="PSUM") as ps1,
                    tc.tile_pool(name="ps2", bufs=4, space="PSUM") as ps2,
                    tc.tile_pool(name="ps3", bufs=2, space="PSUM") as ps3,
                ):
                    for h in range(2):
                        nc.vector.memset(mkl[h][:, 0:1], 1.0)
                    # ---- phase 0: xw = x @ Wx. d-outer (4 concurrent PSUM
                    # groups in the conv psc pool) consuming xt/wx chunks as
                    # DMA segments land; the last d-chunk goes per-m so each
                    # chunk's t=0 tanh (read straight from PSUM) and xw copy
                    # overlap the remaining phase-0 matmuls ----
                    pss = [ps2.tile([128, TL, NC_N], F32, tag="psc",
                                    name="psc") for _ in range(HC)]
                    for d in range(DC - 2):
                        for m in range(HC):
                            nc.tensor.matmul(pss[m][:, 0, :],
                                             wx_sb[:, d, m * 128:(m + 1) * 128],
                                             xt_sb[:, d, :],
                                             start=(d == 0), stop=False,
                                             skip_group_check=True)
                    # t=0: h0 = tanh(xw) read straight from the still-open
                    # PSUM group (partial sum after the last wx chunk = xw).
                    # The last two d-chunks go m-major so tanh(m) can fire
                    # after just 2 more matmuls instead of 5.
                    for m in range(HC):
                        for d in (DC - 2, DC - 1):
                            nc.tensor.matmul(pss[m][:, 0, :],
                                             wx_sb[:, d, m * 128:(m + 1) * 128],
                                             xt_sb[:, d, :],
                                             start=False, stop=False,
                                             skip_group_check=True)
                        nc.scalar.activation(hs[m][:, 0, :], pss[m][:, 0, :],
                                             AF.Tanh, bias=bsum_sb[:, m:m + 1])
                    # xw SBUF copies (for steps 2..31) must read the partial
                    # sum before step 1 accumulates on top; they run on DVE
                    # in parallel with the scalar tanh chain
                    for m in range(HC):
                        nc.vector.tensor_copy(xw[m][:], pss[m][:, 0, :])

                    # ---- RNN step 1: keep accumulating Wh@h0 INTO the
                    # phase-0 groups (they already hold xw), so h1 =
                    # tanh(PSUM) with no vector add on the critical path.
                    # Diagonal wavefront: mm(kc, m) needs tanh(t0, kc)
                    # (moving input) and tanh(t0, m) (write-after-read on
                    # pss[m]), i.e. ready at tanh(max(kc, m)); within a wave
                    # emit m-ascending so group m=0 stops first ----
                    for s in range(HC):
                        wave = [(kc, m) for kc in range(HC) for m in range(HC)
                                if max(kc, m) == s]
                        wave.sort(key=lambda km: (km[1], km[0]))
                        for kc, m in wave:
                            nc.tensor.matmul(
                                pss[m][:, 0, :],
                                wh_sb[:, kc, m * 128:(m + 1) * 128],
                                hs[kc][:, 0, :],
                                start=False, stop=(kc == HC - 1),
                                skip_group_check=True)
                    for m in range(HC):
                        nc.scalar.activation(hs[m][:, 1, :], pss[m][:, 0, :],
                                             AF.Tanh, bias=bsum_sb[:, m:m + 1])

                    conv_waveA(0, ps2)
                    for t in (2, 3, 4):
                        rnn_step(t, ps1)
                    for lt in range(L // TL - 1):
                        conv_ltile(lt, ps2, ps3)
                        if lt + 1 < L // TL - 1:
                            conv_waveA(lt + 1, ps2)
                        for t in (TL * lt + 5, TL * lt + 6):
                            if t < L:
                                rnn_step(t, ps1)
                    conv_ltile_last(ps2, ps3)

    _split_excess_waits(nc, limit=1)
    return nc


def _pack_inputs(inputs):
    """Host-side packing into PE-ready layouts (per-core + replicated)."""
    import ml_dtypes
    bf16 = ml_dtypes.bfloat16
    x = np.ascontiguousarray(inputs["h_w_action"].reshape(N, D_IN))
    wx = np.ascontiguousarray(
        inputs["Wx"].reshape(DC, 128, H).transpose(1, 0, 2)).astype(bf16)
    wh = np.ascontiguousarray(
        inputs["Wh"].reshape(HC, 128, H).transpose(1, 0, 2)).astype(bf16)
    wcs = {}
    for k in KS:
        w = inputs[f"conv_w{k}"]                      # (128, 512, k)
        wt = w.transpose(1, 2, 0).reshape(HC, 128, k, 128)
        wcs[k] = np.ascontiguousarray(
            wt.transpose(1, 2, 0, 3)).reshape(128, k, H).astype(bf16)
    wo = np.ascontiguousarray(
        inputs["Wout"].reshape(HC, 128, C).transpose(1, 0, 2)
    ).reshape(128, HC * C).astype(bf16)
    bsum = np.ascontiguousarray(
        (inputs["bx"] + inputs["bh"]).reshape(HC, 128).T)
    cb = np.ascontiguousarray(np.concatenate(
        [inputs[f"conv_b{k}"] for k in KS]).reshape(HC, 128).T)
    bout = np.ascontiguousarray(np.broadcast_to(inputs["bout"].reshape(1, C), (128, C)))

    in_maps = []
    for c in range(NCORES):
        xs = x[c * NC_N:(c + 1) * NC_N]               # (256, 1536)
        xt = np.ascontiguousarray(
            xs.T.reshape(DC, 128, NC_N).transpose(1, 0, 2)).astype(bf16)
        m = {"xt": xt, "wx": wx, "wh": wh, "wo": wo,
             "bsum": bsum, "cb": cb, "bout": bout}
        for k in KS:
            m[f"wc{k}"] = wcs[k]
        in_maps.append(m)
    return in_maps


_NC_CACHE = {}
_RUNNER_CACHE = {}


def _make_runner(nc):
    """Persistent jitted PJRT runner (mirrors bass2jax.run_bass_via_pjrt's
    multi-core path) so repeat kernel() calls skip re-tracing."""
    import jax
    from jax.sharding import Mesh, PartitionSpec
    try:
        from jax.experimental.shard_map import shard_map
    except ImportError:
        from jax import shard_map
    from concourse import bass2jax

    bass2jax.install_neuronx_cc_hook()
    partition_name = (nc.partition_id_tensor.name
                      if nc.partition_id_tensor else None)
    in_names, out_names, out_avals, zero_outs = [], [], [], []
    for alloc in nc.m.functions[0].allocations:
        if not isinstance(alloc, mybir.MemoryLocationSet):
            continue
        name = alloc.memorylocations[0].name
        if alloc.kind == "ExternalInput":
            if name != partition_name:
                in_names.append(name)
        elif alloc.kind == "ExternalOutput":
            shape = tuple(alloc.tensor_shape)
            dtype = mybir.dt.np(alloc.dtype)
            out_names.append(name)
            out_avals.append(jax.core.ShapedArray(shape, dtype))
            zero_outs.append(np.zeros(shape, dtype))
    n_params, n_outs = len(in_names), len(out_avals)
    all_in_names = list(in_names) + list(out_names)
    if partition_name is not None:
        all_in_names.append(partition_name)

    def _body(*args):
        operands = list(args)
        if partition_name is not None:
            operands.append(bass2jax.partition_id_tensor())
        return tuple(bass2jax._bass_exec_p.bind(
            *operands,
            out_avals=tuple(out_avals),
            in_names=tuple(all_in_names),
            out_names=tuple(out_names),
            lowering_input_output_aliases=(),
            sim_require_finite=True,
            sim_require_nnan=True,
            nc=nc,
        ))

    devices = jax.devices()[:NCORES]
    mesh = Mesh(np.asarray(devices), ("core",))
    in_specs = (PartitionSpec("core"),) * (n_params + n_outs)
    out_specs = (PartitionSpec("core"),) * n_outs
    donate = tuple(range(n_params, n_params + n_outs))
    sharded = jax.jit(
        shard_map(_body, mesh=mesh, in_specs=in_specs, out_specs=out_specs,
                  check_rep=False),
        donate_argnums=donate, keep_unused=True)

    def call(in_maps):
        concat_in = [np.concatenate([np.asarray(in_maps[c][nm])
                                     for c in range(NCORES)], axis=0)
                     for nm in in_names]
        zeros = [np.zeros((NCORES * z.shape[0], *z.shape[1:]), z.dtype)
                 for z in zero_outs]
        out_arrs = sharded(*concat_in, *zeros)
        oidx = out_names.index("out")
        full = np.asarray(out_arrs[oidx])
        return full.reshape(NCORES, NC_N, L, C)

    return call


def kernel(**inputs) -> np.ndarray:
    inputs = {k: np.asarray(v, dtype=np.float32) for k, v in inputs.items()}
    prelu_a = float(np.asarray(inputs["prelu_a"]))
    bias_free = not np.any(inputs["bout"])
    key = (prelu_a, 1, bias_free)
    if key not in _NC_CACHE:
        _NC_CACHE[key] = build_nc(prelu_a, rep=1, bias_free=bias_free)
    nc = _NC_CACHE[key]
    in_maps = _pack_inputs(inputs)
    try:
        if key not in _RUNNER_CACHE:
            _RUNNER_CACHE[key] = _make_runner(nc)
        out = _RUNNER_CACHE[key](in_maps)
    except Exception:
        res = run_bass_kernel_spmd(nc, in_maps, core_ids=list(range(NCORES)))
        out = np.stack([res.results[c]["out"] for c in range(NCORES)], axis=0)
    return out.reshape(E, S, L, C).astype(np.float32)

